# revision 8
# baseline (speedup 1.0000x reference)
"""AttentionBlock kernel for 8 TRN2 NeuronCores.

Problem (hardcoded shapes): x (4, 256, 64, 64) f32, w_qkv (768, 256),
w_out (256, 256), b_out (256,). heads=4, d=64, seq=hw=4096.

Sharding: 16 independent (batch, head) attention units -> 8 cores,
core i handles batch i//2, head-pair i%2 (2 heads). Each core computes
its batch's qkv rows for its heads, flash-style attention (scores kept
transposed: j on partitions, q on free dim; softmax denominator via a
ones-column appended to V), and a partial output projection over its
128 local channels. Host sums the two partial projections per batch and
adds x + b_out (the cheap rank-1 epilogue).

All matmuls run in bf16 (f32 PSUM accumulate); softmax exp runs on
ScalarE in f32 from PSUM, grouped over 3 PSUM banks per instruction to
amortize ACT overhead. Weights are pre-transposed/sliced on host so the
device does no layout fixups.
"""

import os
import sys
import types

import numpy as np
import ml_dtypes

# The agent image's antenv package lacks axon_hooks; the axon boot code
# degrades silently and run_bass_kernel_spmd(trace=True) then crashes on
# import. Pre-register the module so the boot can install the NTFF hook.
# Harmless when tracing is off.
if "antenv.axon_hooks" not in sys.modules:
    _m = types.ModuleType("antenv.axon_hooks")
    _m._hook = None

    def _set(h, _m=_m):
        _m._hook = h

    def _get(_m=_m):
        return _m._hook

    _m.set_axon_ntff_profile_hook = _set
    _m.get_axon_ntff_profile_hook = _get
    sys.modules["antenv.axon_hooks"] = _m
    # The axon boot (sitecustomize) runs before this module exists and
    # skips hook registration; re-derive the ctypes hook it would have
    # installed so trace=True can capture NTFF profiles.
    try:
        from trn_agent_boot.trn_boot import _ntff_profile_via_ctypes
        _m._hook = _ntff_profile_via_ctypes("/opt/axon/libaxon_pjrt.so")
    except Exception:
        pass

B = 4
C = 256
HW = 4096
HEADS = 4
D = 64
SCALE = D ** -0.5
N_CORES = 8
QB = 512          # q positions per block
NQB = HW // QB    # 8
JC = 128          # j positions per chunk (scores-matmul output partitions)
NJC = HW // JC    # 32
VROW = 2 * (D + 1)  # per-j-chunk v layout: [v_h0(64) | 1 | v_h1(64) | 1]
# exp groups: 3 PSUM banks per ACT instruction (10x3 + 1x2 = 32 chunks)
GROUPS = [3] * 10 + [2]

_BF16 = ml_dtypes.bfloat16

_CACHE = {}
LAST_RESULTS = None


def _build():
    import concourse.bass as bass
    import concourse.tile as tile
    from concourse import bacc, mybir

    f32 = mybir.dt.float32
    bf16 = mybir.dt.bfloat16
    Exp = mybir.ActivationFunctionType.Exp

    nc = bacc.Bacc("TRN2", target_bir_lowering=False, debug=False,
                   enable_asserts=False)

    x_d = nc.dram_tensor("x", [C, HW], bf16, kind="ExternalInput").ap()
    wqkT_d = nc.dram_tensor("wqkT", [C, 2 * 128], bf16, kind="ExternalInput").ap()
    wvT_d = nc.dram_tensor("wvT", [C, 128], bf16, kind="ExternalInput").ap()
    woT_d = nc.dram_tensor("woT", [128, C], bf16, kind="ExternalInput").ap()
    out_d = nc.dram_tensor("out", [C, HW], f32, kind="ExternalOutput").ap()

    with tile.TileContext(nc) as tc:
        with (
            tc.tile_pool(name="big", bufs=1) as big,
            tc.tile_pool(name="attn", bufs=3) as attnp,
            tc.tile_pool(name="small", bufs=2) as small,
            tc.tile_pool(name="psc", bufs=2, space="PSUM") as psc,
            tc.tile_pool(name="pout", bufs=1, space="PSUM") as pout,
            tc.tile_pool(name="pbc", bufs=1, space="PSUM") as pbc,
        ):
            # ---- load inputs ----
            xb = []
            for kc in range(2):
                t = big.tile([128, HW], bf16, name=f"xb{kc}", tag=f"xb{kc}")
                nc.sync.dma_start(t[:], x_d[kc * 128:(kc + 1) * 128, :])
                xb.append(t)
            wqkT = []
            for kc in range(2):
                t = big.tile([128, 256], bf16, name=f"wqkT{kc}", tag=f"wqkT{kc}")
                nc.sync.dma_start(t[:], wqkT_d[kc * 128:(kc + 1) * 128, :])
                wqkT.append(t)
            wvT = []
            for kc in range(2):
                t = big.tile([128, 128], bf16, name=f"wvT{kc}", tag=f"wvT{kc}")
                nc.sync.dma_start(t[:], wvT_d[kc * 128:(kc + 1) * 128, :])
                wvT.append(t)
            woT = big.tile([128, 256], bf16, name="woT", tag="woT")
            nc.sync.dma_start(woT[:], woT_d[:, :])

            ones_sb = big.tile([1, D], f32, name="ones_sb", tag="ones_sb")
            nc.vector.memset(ones_sb[:], 1.0)

            q_sb = big.tile([128, HW], bf16, name="q_sb", tag="q_sb")
            k_sb = big.tile([128, HW], bf16, name="k_sb", tag="k_sb")
            v_sb = big.tile([128, NJC * VROW], bf16, name="v_sb", tag="v_sb")
            o_sb = big.tile([128, HW], bf16, name="o_sb", tag="o_sb")

            # ---- qkv projections ----
            # q_sb/k_sb: (2 heads * 64 chan, pos);  m=0 -> q rows, m=1 -> k rows
            for m in range(2):
                dest = q_sb if m == 0 else k_sb
                for nb in range(NQB):
                    ps = psc.tile([128, QB], f32, name="ps_qk", tag="psc")
                    for kc in range(2):
                        nc.tensor.matmul(
                            ps[:],
                            lhsT=wqkT[kc][:, m * 128:(m + 1) * 128],
                            rhs=xb[kc][:, nb * QB:(nb + 1) * QB],
                            start=(kc == 0), stop=(kc == 1),
                        )
                    nc.vector.tensor_copy(dest[:, nb * QB:(nb + 1) * QB], ps[:])

            # v transposed: per j-chunk (128 pos, [v_h0|1|v_h1|1])
            nc.vector.memset(v_sb[:], 1.0)
            for pc in range(NJC):
                ps = psc.tile([128, 128], f32, name="ps_v", tag="psc")
                for kc in range(2):
                    nc.tensor.matmul(
                        ps[:],
                        lhsT=xb[kc][:, pc * 128:(pc + 1) * 128],
                        rhs=wvT[kc][:],
                        start=(kc == 0), stop=(kc == 1),
                    )
                base = pc * VROW
                nc.vector.tensor_copy(v_sb[:, base:base + D], ps[:, 0:D])
                nc.vector.tensor_copy(
                    v_sb[:, base + D + 1:base + 2 * D + 1], ps[:, D:2 * D])

            # ---- attention ----
            for h in range(2):
                hp = h * D          # partition offset of this head in q/k/o
                vo = h * (D + 1)    # free offset of this head in v chunk
                for qb in range(NQB):
                    out_ps = pout.tile([D + 1, QB], f32, name="out_ps",
                                       tag="pout")
                    j = 0
                    for gsz in GROUPS:
                        s_ps = psc.tile([128, 3 * QB], f32, name="s_ps",
                                        tag="psc")
                        for t in range(gsz):
                            nc.tensor.matmul(
                                s_ps[:, t * QB:(t + 1) * QB],
                                lhsT=k_sb[hp:hp + D,
                                          (j + t) * JC:(j + t + 1) * JC],
                                rhs=q_sb[hp:hp + D, qb * QB:(qb + 1) * QB],
                                start=True, stop=True,
                            )
                        a_sb = attnp.tile([128, 3 * QB], bf16, name="a_sb",
                                          tag="attn")
                        nc.scalar.activation(
                            a_sb[:, 0:gsz * QB], s_ps[:, 0:gsz * QB],
                            Exp, scale=SCALE)
                        for t in range(gsz):
                            jj = j + t
                            nc.tensor.matmul(
                                out_ps[:],
                                lhsT=v_sb[:, jj * VROW + vo:
                                          jj * VROW + vo + D + 1],
                                rhs=a_sb[:, t * QB:(t + 1) * QB],
                                start=(jj == 0), stop=(jj == NJC - 1),
                            )
                        j += gsz
                    recip = small.tile([1, QB], f32, name="recip", tag="recip")
                    nc.vector.reciprocal(recip[:], out_ps[D:D + 1, :])
                    # broadcast recip along partitions via a K=1 matmul
                    bc = pbc.tile([D, QB], f32, name="bc", tag="pbc")
                    nc.tensor.matmul(bc[:], lhsT=ones_sb[:], rhs=recip[:],
                                     start=True, stop=True)
                    bc_sb = small.tile([D, QB], f32, name="bc_sb", tag="bc_sb")
                    nc.vector.tensor_copy(bc_sb[:], bc[:])
                    nc.vector.tensor_mul(
                        o_sb[hp:hp + D, qb * QB:(qb + 1) * QB],
                        out_ps[0:D, :], bc_sb[:])

            # ---- output projection (partial: this core's 128 channels) ----
            for nb in range(NQB):
                for m in range(2):
                    ps = psc.tile([128, QB], f32, name="ps_pr", tag="psc")
                    nc.tensor.matmul(
                        ps[:],
                        lhsT=woT[:, m * 128:(m + 1) * 128],
                        rhs=o_sb[:, nb * QB:(nb + 1) * QB],
                        start=True, stop=True,
                    )
                    st = small.tile([128, QB], f32, name="st", tag="st")
                    nc.vector.tensor_copy(st[:], ps[:])
                    nc.sync.dma_start(
                        out_d[m * 128:(m + 1) * 128, nb * QB:(nb + 1) * QB],
                        st[:])

    nc.compile()
    return nc


def kernel(x, w_qkv, w_out, b_out):
    from concourse.bass_utils import run_bass_kernel_spmd
    global LAST_RESULTS

    if "nc" not in _CACHE:
        _CACHE["nc"] = _build()
    nc = _CACHE["nc"]

    x = np.ascontiguousarray(np.asarray(x, dtype=np.float32))
    w_qkv = np.asarray(w_qkv, dtype=np.float32)
    w_out = np.asarray(w_out, dtype=np.float32)
    b_out = np.asarray(b_out, dtype=np.float32)

    xf = x.reshape(B, C, HW)
    in_maps = []
    for core in range(N_CORES):
        bi, hp = divmod(core, 2)
        # rows of w_qkv for this core's two heads: q block then k block
        q_rows = w_qkv[0 * C + hp * 128: 0 * C + hp * 128 + 128]
        k_rows = w_qkv[1 * C + hp * 128: 1 * C + hp * 128 + 128]
        v_rows = w_qkv[2 * C + hp * 128: 2 * C + hp * 128 + 128]
        wqkT = np.concatenate([q_rows, k_rows], axis=0).T  # (256, 256)
        wvT = v_rows.T                                     # (256, 128)
        woT = w_out[:, hp * 128:(hp + 1) * 128].T          # (128, 256)
        in_maps.append({
            "x": np.ascontiguousarray(xf[bi]).astype(_BF16),
            "wqkT": np.ascontiguousarray(wqkT).astype(_BF16),
            "wvT": np.ascontiguousarray(wvT).astype(_BF16),
            "woT": np.ascontiguousarray(woT).astype(_BF16),
        })

    trace = bool(int(os.environ.get("KERNEL_TRACE", "0")))
    print("kernel: program built, launching spmd run", flush=True)
    LAST_RESULTS = run_bass_kernel_spmd(
        nc, in_maps, core_ids=list(range(N_CORES)), trace=trace)

    out = np.empty((B, C, HW), dtype=np.float32)
    for bi in range(B):
        p = LAST_RESULTS.results[2 * bi]["out"] + \
            LAST_RESULTS.results[2 * bi + 1]["out"]
        out[bi] = xf[bi] + p + b_out[:, None]
    return out.reshape(B, C, 64, 64)


# revision 10
# speedup vs baseline: 1.0070x; 1.0070x over previous
"""AttentionBlock kernel for 8 TRN2 NeuronCores.

Problem (hardcoded shapes): x (4, 256, 64, 64) f32, w_qkv (768, 256),
w_out (256, 256), b_out (256,). heads=4, d=64, seq=hw=4096.

Sharding: 16 independent (batch, head) attention units -> 8 cores,
core i handles batch i//2, head-pair i%2 (2 heads). Each core computes
its batch's qkv rows for its heads, flash-style attention (scores kept
transposed: j on partitions, q on free dim; softmax denominator via a
ones-column appended to V), and a partial output projection over its
128 local channels. Host sums the two partial projections per batch and
adds x + b_out (the cheap rank-1 epilogue).

All matmuls run in bf16 (f32 PSUM accumulate); softmax exp runs on
ScalarE in f32 from PSUM, grouped over 3 PSUM banks per instruction to
amortize ACT overhead. Weights are pre-transposed/sliced on host so the
device does no layout fixups.
"""

import os
import sys
import types

import numpy as np
import ml_dtypes

# The agent image's antenv package lacks axon_hooks; the axon boot code
# degrades silently and run_bass_kernel_spmd(trace=True) then crashes on
# import. Pre-register the module so the boot can install the NTFF hook.
# Harmless when tracing is off.
if "antenv.axon_hooks" not in sys.modules:
    _m = types.ModuleType("antenv.axon_hooks")
    _m._hook = None

    def _set(h, _m=_m):
        _m._hook = h

    def _get(_m=_m):
        return _m._hook

    _m.set_axon_ntff_profile_hook = _set
    _m.get_axon_ntff_profile_hook = _get
    sys.modules["antenv.axon_hooks"] = _m
    # The axon boot (sitecustomize) runs before this module exists and
    # skips hook registration; re-derive the ctypes hook it would have
    # installed so trace=True can capture NTFF profiles.
    try:
        from trn_agent_boot.trn_boot import _ntff_profile_via_ctypes
        _m._hook = _ntff_profile_via_ctypes("/opt/axon/libaxon_pjrt.so")
    except Exception:
        pass

B = 4
C = 256
HW = 4096
HEADS = 4
D = 64
SCALE = D ** -0.5
N_CORES = 8
QB = 512          # q positions per block
NQB = HW // QB    # 8
JC = 128          # j positions per chunk (scores-matmul output partitions)
NJC = HW // JC    # 32
VROW = 2 * (D + 1)  # per-j-chunk v layout: [v_h0(64) | 1 | v_h1(64) | 1]
# exp groups: 3 PSUM banks per ACT instruction (10x3 + 1x2 = 32 chunks)
GROUPS = [3] * 10 + [2]

_BF16 = ml_dtypes.bfloat16

_CACHE = {}
LAST_RESULTS = None


def _build():
    import concourse.bass as bass
    import concourse.tile as tile
    from concourse import bacc, mybir

    f32 = mybir.dt.float32
    bf16 = mybir.dt.bfloat16
    Exp = mybir.ActivationFunctionType.Exp

    nc = bacc.Bacc("TRN2", target_bir_lowering=False, debug=False,
                   enable_asserts=False)

    x_d = nc.dram_tensor("x", [C, HW], bf16, kind="ExternalInput").ap()
    wqkT_d = nc.dram_tensor("wqkT", [C, 2 * 128], bf16, kind="ExternalInput").ap()
    wvT_d = nc.dram_tensor("wvT", [C, 128], bf16, kind="ExternalInput").ap()
    woT_d = nc.dram_tensor("woT", [128, C], bf16, kind="ExternalInput").ap()
    out_d = nc.dram_tensor("out", [C, HW], f32, kind="ExternalOutput").ap()

    with tile.TileContext(nc) as tc:
        with (
            tc.tile_pool(name="big", bufs=1) as big,
            tc.tile_pool(name="attn", bufs=3) as attnp,
            tc.tile_pool(name="small", bufs=2) as small,
            tc.tile_pool(name="psc", bufs=2, space="PSUM") as psc,
            tc.tile_pool(name="pout", bufs=1, space="PSUM") as pout,
        ):
            # ---- load inputs ----
            xb = []
            for kc in range(2):
                t = big.tile([128, HW], bf16, name=f"xb{kc}", tag=f"xb{kc}")
                nc.sync.dma_start(t[:], x_d[kc * 128:(kc + 1) * 128, :])
                xb.append(t)
            wqkT = []
            for kc in range(2):
                t = big.tile([128, 256], bf16, name=f"wqkT{kc}", tag=f"wqkT{kc}")
                nc.sync.dma_start(t[:], wqkT_d[kc * 128:(kc + 1) * 128, :])
                wqkT.append(t)
            wvT = []
            for kc in range(2):
                t = big.tile([128, 128], bf16, name=f"wvT{kc}", tag=f"wvT{kc}")
                nc.sync.dma_start(t[:], wvT_d[kc * 128:(kc + 1) * 128, :])
                wvT.append(t)
            woT = big.tile([128, 256], bf16, name="woT", tag="woT")
            nc.sync.dma_start(woT[:], woT_d[:, :])

            ones_sb = big.tile([1, D], f32, name="ones_sb", tag="ones_sb")
            nc.vector.memset(ones_sb[:], 1.0)

            q_sb = big.tile([128, HW], bf16, name="q_sb", tag="q_sb")
            k_sb = big.tile([128, HW], bf16, name="k_sb", tag="k_sb")
            v_sb = big.tile([128, NJC * VROW], bf16, name="v_sb", tag="v_sb")
            o_sb = big.tile([128, HW], bf16, name="o_sb", tag="o_sb")

            # ---- qkv projections ----
            # q_sb/k_sb: (2 heads * 64 chan, pos);  m=0 -> q rows, m=1 -> k rows
            for m in range(2):
                dest = q_sb if m == 0 else k_sb
                for nb in range(NQB):
                    ps = psc.tile([128, QB], f32, name="ps_qk", tag="psc")
                    for kc in range(2):
                        nc.tensor.matmul(
                            ps[:],
                            lhsT=wqkT[kc][:, m * 128:(m + 1) * 128],
                            rhs=xb[kc][:, nb * QB:(nb + 1) * QB],
                            start=(kc == 0), stop=(kc == 1),
                        )
                    nc.vector.tensor_copy(dest[:, nb * QB:(nb + 1) * QB], ps[:])

            # v transposed: per j-chunk (128 pos, [v_h0|1|v_h1|1])
            nc.vector.memset(v_sb[:], 1.0)
            for pc in range(NJC):
                ps = psc.tile([128, 128], f32, name="ps_v", tag="psc")
                for kc in range(2):
                    nc.tensor.matmul(
                        ps[:],
                        lhsT=xb[kc][:, pc * 128:(pc + 1) * 128],
                        rhs=wvT[kc][:],
                        start=(kc == 0), stop=(kc == 1),
                    )
                base = pc * VROW
                nc.vector.tensor_copy(v_sb[:, base:base + D], ps[:, 0:D])
                nc.vector.tensor_copy(
                    v_sb[:, base + D + 1:base + 2 * D + 1], ps[:, D:2 * D])

            # ---- attention ----
            # Heads interleaved per q-block: adjacent score matmuls use
            # disjoint PE row groups (h0 rows 0-63, h1 rows 64-127) and run
            # concurrently. Stream index s -> (j, h) = (s // 2, s % 2).
            NS = 2 * NJC
            for qb in range(NQB):
                out_ps = [
                    pout.tile([D + 1, QB], f32, name=f"out_ps{h}",
                              tag=f"pout{h}")
                    for h in range(2)
                ]
                s = 0
                while s < NS:
                    gsz = min(3, NS - s)
                    s_ps = psc.tile([128, 3 * QB], f32, name="s_ps", tag="psc")
                    for t in range(gsz):
                        j, h = divmod(s + t, 2)
                        hp = h * D
                        nc.tensor.matmul(
                            s_ps[:, t * QB:(t + 1) * QB],
                            lhsT=k_sb[hp:hp + D, j * JC:(j + 1) * JC],
                            rhs=q_sb[hp:hp + D, qb * QB:(qb + 1) * QB],
                            start=True, stop=True,
                        )
                    a_sb = attnp.tile([128, 3 * QB], bf16, name="a_sb",
                                      tag="attn")
                    nc.scalar.activation(
                        a_sb[:, 0:gsz * QB], s_ps[:, 0:gsz * QB],
                        Exp, scale=SCALE)
                    for t in range(gsz):
                        j, h = divmod(s + t, 2)
                        vo = h * (D + 1)
                        nc.tensor.matmul(
                            out_ps[h][:],
                            lhsT=v_sb[:, j * VROW + vo:j * VROW + vo + D + 1],
                            rhs=a_sb[:, t * QB:(t + 1) * QB],
                            start=(j == 0), stop=(j == NJC - 1),
                        )
                    s += gsz
                for h in range(2):
                    hp = h * D
                    # free the PSUM bank fast: copy out+denom to SBUF, then
                    # normalize from the copy off the accumulation path
                    o65 = small.tile([D + 1, QB], f32, name="o65", tag="o65")
                    nc.vector.tensor_copy(o65[:], out_ps[h][:])
                    recip = small.tile([1, QB], f32, name="recip", tag="recip")
                    nc.vector.reciprocal(recip[:], o65[D:D + 1, :])
                    # broadcast recip along partitions via a K=1 matmul into
                    # the just-freed out bank
                    bc = pout.tile([D, QB], f32, name=f"bc{h}", tag=f"pout{h}")
                    nc.tensor.matmul(bc[:], lhsT=ones_sb[:], rhs=recip[:],
                                     start=True, stop=True)
                    nc.vector.tensor_mul(
                        o_sb[hp:hp + D, qb * QB:(qb + 1) * QB],
                        o65[0:D, :], bc[:])

            # ---- output projection (partial: this core's 128 channels) ----
            for nb in range(NQB):
                for m in range(2):
                    ps = psc.tile([128, QB], f32, name="ps_pr", tag="psc")
                    nc.tensor.matmul(
                        ps[:],
                        lhsT=woT[:, m * 128:(m + 1) * 128],
                        rhs=o_sb[:, nb * QB:(nb + 1) * QB],
                        start=True, stop=True,
                    )
                    st = small.tile([128, QB], f32, name="st", tag="st")
                    nc.vector.tensor_copy(st[:], ps[:])
                    nc.sync.dma_start(
                        out_d[m * 128:(m + 1) * 128, nb * QB:(nb + 1) * QB],
                        st[:])

    nc.compile()
    return nc


def kernel(x, w_qkv, w_out, b_out):
    from concourse.bass_utils import run_bass_kernel_spmd
    global LAST_RESULTS

    if "nc" not in _CACHE:
        _CACHE["nc"] = _build()
    nc = _CACHE["nc"]

    x = np.ascontiguousarray(np.asarray(x, dtype=np.float32))
    w_qkv = np.asarray(w_qkv, dtype=np.float32)
    w_out = np.asarray(w_out, dtype=np.float32)
    b_out = np.asarray(b_out, dtype=np.float32)

    xf = x.reshape(B, C, HW)
    in_maps = []
    for core in range(N_CORES):
        bi, hp = divmod(core, 2)
        # rows of w_qkv for this core's two heads: q block then k block
        q_rows = w_qkv[0 * C + hp * 128: 0 * C + hp * 128 + 128]
        k_rows = w_qkv[1 * C + hp * 128: 1 * C + hp * 128 + 128]
        v_rows = w_qkv[2 * C + hp * 128: 2 * C + hp * 128 + 128]
        wqkT = np.concatenate([q_rows, k_rows], axis=0).T  # (256, 256)
        wvT = v_rows.T                                     # (256, 128)
        woT = w_out[:, hp * 128:(hp + 1) * 128].T          # (128, 256)
        in_maps.append({
            "x": np.ascontiguousarray(xf[bi]).astype(_BF16),
            "wqkT": np.ascontiguousarray(wqkT).astype(_BF16),
            "wvT": np.ascontiguousarray(wvT).astype(_BF16),
            "woT": np.ascontiguousarray(woT).astype(_BF16),
        })

    trace = bool(int(os.environ.get("KERNEL_TRACE", "0")))
    print("kernel: program built, launching spmd run", flush=True)
    LAST_RESULTS = run_bass_kernel_spmd(
        nc, in_maps, core_ids=list(range(N_CORES)), trace=trace)

    out = np.empty((B, C, HW), dtype=np.float32)
    for bi in range(B):
        p = LAST_RESULTS.results[2 * bi]["out"] + \
            LAST_RESULTS.results[2 * bi + 1]["out"]
        out[bi] = xf[bi] + p + b_out[:, None]
    return out.reshape(B, C, 64, 64)


# revision 14
# speedup vs baseline: 1.3705x; 1.3610x over previous
"""AttentionBlock kernel for 8 TRN2 NeuronCores.

Problem (hardcoded shapes): x (4, 256, 64, 64) f32, w_qkv (768, 256),
w_out (256, 256), b_out (256,). heads=4, d=64, seq=hw=4096.

Sharding: 16 independent (batch, head) attention units -> 8 cores,
core i handles batch i//2, head-pair i%2 (2 heads). Each core computes
its batch's qkv rows for its heads, flash-style attention (scores kept
transposed: j on partitions, q on free dim; softmax denominator via a
ones-column appended to V), and per-head partial output projections of
the UNNORMALIZED attention output. The softmax denominator rows ship to
the host, which applies the per-position division (it commutes with the
channel-mixing projection), sums partial projections, and adds
x + b_out.

All matmuls run in bf16 (f32 PSUM accumulate); softmax exp runs on
ScalarE in f32 from PSUM, grouped over 3 PSUM banks per instruction to
amortize ACT overhead. The two heads interleave per q-block so adjacent
score matmuls land on disjoint PE row groups and run concurrently.
Weights are pre-transposed/sliced on host so the device does no layout
fixups.
"""

import os
import sys
import types

import numpy as np
import ml_dtypes

# The agent image's antenv package lacks axon_hooks; the axon boot code
# degrades silently and run_bass_kernel_spmd(trace=True) then crashes on
# import. Pre-register the module so the boot can install the NTFF hook.
# Harmless when tracing is off.
if "antenv.axon_hooks" not in sys.modules:
    _m = types.ModuleType("antenv.axon_hooks")
    _m._hook = None

    def _set(h, _m=_m):
        _m._hook = h

    def _get(_m=_m):
        return _m._hook

    _m.set_axon_ntff_profile_hook = _set
    _m.get_axon_ntff_profile_hook = _get
    sys.modules["antenv.axon_hooks"] = _m
    # The axon boot (sitecustomize) runs before this module exists and
    # skips hook registration; re-derive the ctypes hook it would have
    # installed so trace=True can capture NTFF profiles.
    try:
        from trn_agent_boot.trn_boot import _ntff_profile_via_ctypes
        _m._hook = _ntff_profile_via_ctypes("/opt/axon/libaxon_pjrt.so")
    except Exception:
        pass

B = 4
C = 256
HW = 4096
HEADS = 4
D = 64
SCALE = D ** -0.5
N_CORES = 8
QB = 512          # q positions per block
NQB = HW // QB    # 8
JC = 128          # j positions per chunk (scores-matmul output partitions)
NJC = HW // JC    # 32
VROW = 2 * (D + 1)  # per-j-chunk v layout: [v_h0(64) | 1 | v_h1(64) | 1]

_BF16 = ml_dtypes.bfloat16

_CACHE = {}
LAST_RESULTS = None


def _build():
    import concourse.bass as bass
    import concourse.tile as tile
    from concourse import bacc, mybir

    f32 = mybir.dt.float32
    bf16 = mybir.dt.bfloat16
    Exp = mybir.ActivationFunctionType.Exp

    nc = bacc.Bacc("TRN2", target_bir_lowering=False, debug=False,
                   enable_asserts=False)

    x_d = nc.dram_tensor("x", [C, HW], bf16, kind="ExternalInput").ap()
    wqkT_d = nc.dram_tensor("wqkT", [C, 2 * 128], bf16, kind="ExternalInput").ap()
    wvT_d = nc.dram_tensor("wvT", [C, 128], bf16, kind="ExternalInput").ap()
    # woT rows: head dim d (64); cols: [h0 out-chans (256) | h1 out-chans]
    woT_d = nc.dram_tensor("woT", [D, 2 * C], bf16, kind="ExternalInput").ap()
    out0_d = nc.dram_tensor("out0", [C, HW], f32, kind="ExternalOutput").ap()
    out1_d = nc.dram_tensor("out1", [C, HW], f32, kind="ExternalOutput").ap()
    den_d = nc.dram_tensor("den", [2, HW], f32, kind="ExternalOutput").ap()

    with tile.TileContext(nc) as tc:
        with (
            tc.tile_pool(name="big", bufs=1) as big,
            tc.tile_pool(name="attn", bufs=3) as attnp,
            tc.tile_pool(name="small", bufs=2) as small,
            tc.tile_pool(name="psc", bufs=2, space="PSUM") as psc,
            tc.tile_pool(name="pout", bufs=1, space="PSUM") as pout,
        ):
            # ---- load inputs ----
            xb = []
            for kc in range(2):
                t = big.tile([128, HW], bf16, name=f"xb{kc}", tag=f"xb{kc}")
                nc.sync.dma_start(t[:], x_d[kc * 128:(kc + 1) * 128, :])
                xb.append(t)
            wqkT = []
            for kc in range(2):
                t = big.tile([128, 256], bf16, name=f"wqkT{kc}", tag=f"wqkT{kc}")
                nc.sync.dma_start(t[:], wqkT_d[kc * 128:(kc + 1) * 128, :])
                wqkT.append(t)
            wvT = []
            for kc in range(2):
                t = big.tile([128, 128], bf16, name=f"wvT{kc}", tag=f"wvT{kc}")
                nc.sync.dma_start(t[:], wvT_d[kc * 128:(kc + 1) * 128, :])
                wvT.append(t)
            woT = big.tile([D, 2 * C], bf16, name="woT", tag="woT")
            nc.sync.dma_start(woT[:], woT_d[:, :])

            q_sb = big.tile([128, HW], bf16, name="q_sb", tag="q_sb")
            k_sb = big.tile([128, HW], bf16, name="k_sb", tag="k_sb")
            v_sb = big.tile([128, NJC * VROW], bf16, name="v_sb", tag="v_sb")
            # unnormalized per-head attention output (d on partitions)
            oh_sb = [big.tile([D, HW], bf16, name=f"oh{h}", tag=f"oh{h}")
                     for h in range(2)]
            den_sb = [big.tile([1, HW], f32, name=f"den_sb{h}",
                               tag=f"den_sb{h}") for h in range(2)]

            # ---- qkv projections ----
            # q_sb/k_sb: (2 heads * 64 chan, pos);  m=0 -> q rows, m=1 -> k
            for m in range(2):
                dest = q_sb if m == 0 else k_sb
                for nb in range(NQB):
                    ps = psc.tile([128, QB], f32, name="ps_qk", tag="psc")
                    for kc in range(2):
                        nc.tensor.matmul(
                            ps[:],
                            lhsT=wqkT[kc][:, m * 128:(m + 1) * 128],
                            rhs=xb[kc][:, nb * QB:(nb + 1) * QB],
                            start=(kc == 0), stop=(kc == 1),
                        )
                    nc.vector.tensor_copy(dest[:, nb * QB:(nb + 1) * QB], ps[:])

            # v transposed: per j-chunk (128 pos, [v_h0|1|v_h1|1])
            nc.vector.memset(v_sb[:], 1.0)
            for pc in range(NJC):
                ps = psc.tile([128, 128], f32, name="ps_v", tag="psc")
                for kc in range(2):
                    nc.tensor.matmul(
                        ps[:],
                        lhsT=xb[kc][:, pc * 128:(pc + 1) * 128],
                        rhs=wvT[kc][:],
                        start=(kc == 0), stop=(kc == 1),
                    )
                base = pc * VROW
                nc.vector.tensor_copy(v_sb[:, base:base + D], ps[:, 0:D])
                nc.vector.tensor_copy(
                    v_sb[:, base + D + 1:base + 2 * D + 1], ps[:, D:2 * D])

            # ---- attention ----
            # Heads interleaved per q-block: adjacent score matmuls use
            # disjoint PE row groups (h0 rows 0-63, h1 rows 64-127) and run
            # concurrently. Stream index s -> (j, h) = (s // 2, s % 2).
            NS = 2 * NJC

            def emit_proj(qsl):
                # partial projection of a finished q-block; psum from the
                # pout slots (free between q-blocks)
                for h in range(2):
                    od = out0_d if h == 0 else out1_d
                    for m in range(2):
                        ps = psc.tile([128, QB], f32, name=f"ps_pr{h}",
                                      tag="psc")
                        nc.tensor.matmul(
                            ps[:],
                            lhsT=woT[:, h * C + m * 128:h * C + (m + 1) * 128],
                            rhs=oh_sb[h][:, qsl],
                            start=True, stop=True,
                        )
                        st = small.tile([128, QB], f32, name="st", tag="st")
                        nc.vector.tensor_copy(st[:], ps[:])
                        nc.sync.dma_start(od[m * 128:(m + 1) * 128, qsl], st[:])

            prev_qsl = None
            for qb in range(NQB):
                qsl = slice(qb * QB, (qb + 1) * QB)
                out_ps = [
                    pout.tile([D + 1, QB], f32, name=f"out_ps{h}",
                              tag=f"pout{h}")
                    for h in range(2)
                ]
                s = 0
                while s < NS:
                    gsz = min(3, NS - s)
                    s_ps = psc.tile([128, 3 * QB], f32, name="s_ps", tag="psc")
                    for t in range(gsz):
                        j, h = divmod(s + t, 2)
                        hp = h * D
                        nc.tensor.matmul(
                            s_ps[:, t * QB:(t + 1) * QB],
                            lhsT=k_sb[hp:hp + D, j * JC:(j + 1) * JC],
                            rhs=q_sb[hp:hp + D, qsl],
                            start=True, stop=True,
                        )
                    a_sb = attnp.tile([128, 3 * QB], bf16, name="a_sb",
                                      tag="attn")
                    nc.scalar.activation(
                        a_sb[:, 0:gsz * QB], s_ps[:, 0:gsz * QB],
                        Exp, scale=SCALE)
                    for t in range(gsz):
                        j, h = divmod(s + t, 2)
                        vo = h * (D + 1)
                        nc.tensor.matmul(
                            out_ps[h][:],
                            lhsT=v_sb[:, j * VROW + vo:j * VROW + vo + D + 1],
                            rhs=a_sb[:, t * QB:(t + 1) * QB],
                            start=(j == 0), stop=(j == NJC - 1),
                        )
                    s += gsz
                    if s == 3 and prev_qsl is not None:
                        # previous q-block's projection, emitted after this
                        # block's first score group so PE never waits on it
                        emit_proj(prev_qsl)
                # ship unnormalized output + denominator; no PE in this path
                for h in range(2):
                    nc.vector.tensor_copy(oh_sb[h][:, qsl], out_ps[h][0:D, :])
                    nc.vector.tensor_copy(den_sb[h][0:1, qsl],
                                          out_ps[h][D:D + 1, :])
                prev_qsl = qsl
            emit_proj(prev_qsl)

            for h in range(2):
                nc.sync.dma_start(den_d[h:h + 1, :], den_sb[h][0:1, :])

    nc.compile()
    return nc


def kernel(x, w_qkv, w_out, b_out):
    from concourse.bass_utils import run_bass_kernel_spmd
    global LAST_RESULTS

    if "nc" not in _CACHE:
        _CACHE["nc"] = _build()
    nc = _CACHE["nc"]

    x = np.ascontiguousarray(np.asarray(x, dtype=np.float32))
    w_qkv = np.asarray(w_qkv, dtype=np.float32)
    w_out = np.asarray(w_out, dtype=np.float32)
    b_out = np.asarray(b_out, dtype=np.float32)

    xf = x.reshape(B, C, HW)
    in_maps = []
    for core in range(N_CORES):
        bi, hp = divmod(core, 2)
        # rows of w_qkv for this core's two heads: q block then k block
        q_rows = w_qkv[0 * C + hp * 128: 0 * C + hp * 128 + 128]
        k_rows = w_qkv[1 * C + hp * 128: 1 * C + hp * 128 + 128]
        v_rows = w_qkv[2 * C + hp * 128: 2 * C + hp * 128 + 128]
        wqkT = np.concatenate([q_rows, k_rows], axis=0).T  # (256, 256)
        wvT = v_rows.T                                     # (256, 128)
        # woT: (64, 512): rows = head dim, cols = [h0 out-chans | h1]
        woT = np.concatenate(
            [w_out[:, hp * 128 + h * D: hp * 128 + (h + 1) * D].T
             for h in range(2)], axis=1)
        in_maps.append({
            "x": np.ascontiguousarray(xf[bi]).astype(_BF16),
            "wqkT": np.ascontiguousarray(wqkT).astype(_BF16),
            "wvT": np.ascontiguousarray(wvT).astype(_BF16),
            "woT": np.ascontiguousarray(woT).astype(_BF16),
        })

    trace = bool(int(os.environ.get("KERNEL_TRACE", "0")))
    print("kernel: program built, launching spmd run", flush=True)
    LAST_RESULTS = run_bass_kernel_spmd(
        nc, in_maps, core_ids=list(range(N_CORES)), trace=trace)

    out = np.empty((B, C, HW), dtype=np.float32)
    for bi in range(B):
        acc = xf[bi] + b_out[:, None]
        for hp in range(2):
            r = LAST_RESULTS.results[2 * bi + hp]
            den = r["den"]
            acc = acc + r["out0"] / den[0][None, :] + r["out1"] / den[1][None, :]
        out[bi] = acc
    return out.reshape(B, C, 64, 64)


# revision 15
# speedup vs baseline: 1.3762x; 1.0041x over previous
"""AttentionBlock kernel for 8 TRN2 NeuronCores.

Problem (hardcoded shapes): x (4, 256, 64, 64) f32, w_qkv (768, 256),
w_out (256, 256), b_out (256,). heads=4, d=64, seq=hw=4096.

Sharding: 16 independent (batch, head) attention units -> 8 cores,
core i handles batch i//2, head-pair i%2 (2 heads). Each core computes
its batch's qkv rows for its heads, flash-style attention (scores kept
transposed: j on partitions, q on free dim; softmax denominator via a
ones-column appended to V), and per-head partial output projections of
the UNNORMALIZED attention output. The softmax denominator rows ship to
the host, which applies the per-position division (it commutes with the
channel-mixing projection), sums partial projections, and adds
x + b_out.

All matmuls run in bf16 (f32 PSUM accumulate); softmax exp runs on
ScalarE in f32 from PSUM, grouped over 3 PSUM banks per instruction to
amortize ACT overhead. The two heads interleave per q-block so adjacent
score matmuls land on disjoint PE row groups and run concurrently.
Weights are pre-transposed/sliced on host so the device does no layout
fixups.
"""

import os
import sys
import types

import numpy as np
import ml_dtypes

# The agent image's antenv package lacks axon_hooks; the axon boot code
# degrades silently and run_bass_kernel_spmd(trace=True) then crashes on
# import. Pre-register the module so the boot can install the NTFF hook.
# Harmless when tracing is off.
if "antenv.axon_hooks" not in sys.modules:
    _m = types.ModuleType("antenv.axon_hooks")
    _m._hook = None

    def _set(h, _m=_m):
        _m._hook = h

    def _get(_m=_m):
        return _m._hook

    _m.set_axon_ntff_profile_hook = _set
    _m.get_axon_ntff_profile_hook = _get
    sys.modules["antenv.axon_hooks"] = _m
    # The axon boot (sitecustomize) runs before this module exists and
    # skips hook registration; re-derive the ctypes hook it would have
    # installed so trace=True can capture NTFF profiles.
    try:
        from trn_agent_boot.trn_boot import _ntff_profile_via_ctypes
        _m._hook = _ntff_profile_via_ctypes("/opt/axon/libaxon_pjrt.so")
    except Exception:
        pass

B = 4
C = 256
HW = 4096
HEADS = 4
D = 64
SCALE = D ** -0.5
N_CORES = 8
QB = 512          # q positions per block
NQB = HW // QB    # 8
JC = 128          # j positions per chunk (scores-matmul output partitions)
NJC = HW // JC    # 32
VROW = 2 * (D + 1)  # per-j-chunk v layout: [v_h0(64) | 1 | v_h1(64) | 1]

_BF16 = ml_dtypes.bfloat16

_CACHE = {}
LAST_RESULTS = None


def _build():
    import concourse.bass as bass
    import concourse.tile as tile
    from concourse import bacc, mybir

    f32 = mybir.dt.float32
    bf16 = mybir.dt.bfloat16
    Exp = mybir.ActivationFunctionType.Exp

    nc = bacc.Bacc("TRN2", target_bir_lowering=False, debug=False,
                   enable_asserts=False)

    x_d = nc.dram_tensor("x", [C, HW], bf16, kind="ExternalInput").ap()
    wqkT_d = nc.dram_tensor("wqkT", [C, 2 * 128], bf16, kind="ExternalInput").ap()
    wvT_d = nc.dram_tensor("wvT", [C, 128], bf16, kind="ExternalInput").ap()
    # woT rows: head dim d (64); cols: [h0 out-chans (256) | h1 out-chans]
    woT_d = nc.dram_tensor("woT", [D, 2 * C], bf16, kind="ExternalInput").ap()
    out0_d = nc.dram_tensor("out0", [C, HW], f32, kind="ExternalOutput").ap()
    out1_d = nc.dram_tensor("out1", [C, HW], f32, kind="ExternalOutput").ap()
    den_d = nc.dram_tensor("den", [2, HW], f32, kind="ExternalOutput").ap()

    with tile.TileContext(nc) as tc:
        with (
            tc.tile_pool(name="big", bufs=1) as big,
            tc.tile_pool(name="attn", bufs=3) as attnp,
            tc.tile_pool(name="small", bufs=2) as small,
            tc.tile_pool(name="psc", bufs=2, space="PSUM") as psc,
            tc.tile_pool(name="pout", bufs=1, space="PSUM") as pout,
        ):
            # ---- load inputs ----
            xb = []
            for kc in range(2):
                t = big.tile([128, HW], bf16, name=f"xb{kc}", tag=f"xb{kc}")
                nc.sync.dma_start(t[:], x_d[kc * 128:(kc + 1) * 128, :])
                xb.append(t)
            wqkT = []
            for kc in range(2):
                t = big.tile([128, 256], bf16, name=f"wqkT{kc}", tag=f"wqkT{kc}")
                nc.sync.dma_start(t[:], wqkT_d[kc * 128:(kc + 1) * 128, :])
                wqkT.append(t)
            wvT = []
            for kc in range(2):
                t = big.tile([128, 128], bf16, name=f"wvT{kc}", tag=f"wvT{kc}")
                nc.sync.dma_start(t[:], wvT_d[kc * 128:(kc + 1) * 128, :])
                wvT.append(t)
            woT = big.tile([D, 2 * C], bf16, name="woT", tag="woT")
            nc.sync.dma_start(woT[:], woT_d[:, :])

            q_sb = big.tile([128, HW], bf16, name="q_sb", tag="q_sb")
            k_sb = big.tile([128, HW], bf16, name="k_sb", tag="k_sb")
            v_sb = big.tile([128, NJC * VROW], bf16, name="v_sb", tag="v_sb")
            # unnormalized per-head attention output (d on partitions),
            # row 64 carries the softmax denominator (unused by proj)
            oh_sb = [big.tile([D + 1, HW], bf16, name=f"oh{h}", tag=f"oh{h}")
                     for h in range(2)]
            den_sb = [big.tile([1, HW], f32, name=f"den_sb{h}",
                               tag=f"den_sb{h}") for h in range(2)]

            # ---- qkv projections ----
            # q_sb/k_sb: (2 heads * 64 chan, pos);  m=0 -> q rows, m=1 -> k
            for m in range(2):
                dest = q_sb if m == 0 else k_sb
                for nb in range(NQB):
                    ps = psc.tile([128, QB], f32, name="ps_qk", tag="psc")
                    for kc in range(2):
                        nc.tensor.matmul(
                            ps[:],
                            lhsT=wqkT[kc][:, m * 128:(m + 1) * 128],
                            rhs=xb[kc][:, nb * QB:(nb + 1) * QB],
                            start=(kc == 0), stop=(kc == 1),
                        )
                    nc.vector.tensor_copy(dest[:, nb * QB:(nb + 1) * QB], ps[:])

            # v transposed: per j-chunk (128 pos, [v_h0|1|v_h1|1])
            nc.vector.memset(v_sb[:], 1.0)
            for pc in range(NJC):
                ps = psc.tile([128, 128], f32, name="ps_v", tag="psc")
                for kc in range(2):
                    nc.tensor.matmul(
                        ps[:],
                        lhsT=xb[kc][:, pc * 128:(pc + 1) * 128],
                        rhs=wvT[kc][:],
                        start=(kc == 0), stop=(kc == 1),
                    )
                base = pc * VROW
                nc.vector.tensor_copy(v_sb[:, base:base + D], ps[:, 0:D])
                nc.vector.tensor_copy(
                    v_sb[:, base + D + 1:base + 2 * D + 1], ps[:, D:2 * D])

            # ---- attention ----
            # Heads interleaved per q-block: adjacent score matmuls use
            # disjoint PE row groups (h0 rows 0-63, h1 rows 64-127) and run
            # concurrently. Stream index s -> (j, h) = (s // 2, s % 2).
            NS = 2 * NJC

            def emit_proj(qsl):
                # partial projection of a finished q-block; psum from the
                # pout slots (free between q-blocks)
                for h in range(2):
                    od = out0_d if h == 0 else out1_d
                    for m in range(2):
                        ps = psc.tile([128, QB], f32, name=f"ps_pr{h}",
                                      tag="psc")
                        nc.tensor.matmul(
                            ps[:],
                            lhsT=woT[:, h * C + m * 128:h * C + (m + 1) * 128],
                            rhs=oh_sb[h][0:D, qsl],
                            start=True, stop=True,
                        )
                        st = small.tile([128, QB], f32, name="st", tag="st")
                        nc.vector.tensor_copy(st[:], ps[:])
                        nc.sync.dma_start(od[m * 128:(m + 1) * 128, qsl], st[:])

            prev_qsl = None
            for qb in range(NQB):
                qsl = slice(qb * QB, (qb + 1) * QB)
                out_ps = [
                    pout.tile([D + 1, QB], f32, name=f"out_ps{h}",
                              tag=f"pout{h}")
                    for h in range(2)
                ]
                s = 0
                while s < NS:
                    gsz = min(3, NS - s)
                    s_ps = psc.tile([128, 3 * QB], f32, name="s_ps", tag="psc")
                    for t in range(gsz):
                        j, h = divmod(s + t, 2)
                        hp = h * D
                        nc.tensor.matmul(
                            s_ps[:, t * QB:(t + 1) * QB],
                            lhsT=k_sb[hp:hp + D, j * JC:(j + 1) * JC],
                            rhs=q_sb[hp:hp + D, qsl],
                            start=True, stop=True,
                        )
                    a_sb = attnp.tile([128, 3 * QB], bf16, name="a_sb",
                                      tag="attn")
                    nc.scalar.activation(
                        a_sb[:, 0:gsz * QB], s_ps[:, 0:gsz * QB],
                        Exp, scale=SCALE)
                    for t in range(gsz):
                        j, h = divmod(s + t, 2)
                        vo = h * (D + 1)
                        nc.tensor.matmul(
                            out_ps[h][:],
                            lhsT=v_sb[:, j * VROW + vo:j * VROW + vo + D + 1],
                            rhs=a_sb[:, t * QB:(t + 1) * QB],
                            start=(j == 0), stop=(j == NJC - 1),
                        )
                    s += gsz
                    if s == 3 and prev_qsl is not None:
                        # previous q-block's projection, emitted after this
                        # block's first score group so PE never waits on it
                        emit_proj(prev_qsl)
                # ship unnormalized output + denominator. The 65-row copy
                # runs on ScalarE (idle at block boundaries, fast PSUM port)
                # and the f32 denominator copy on VectorE in parallel, so
                # the accumulator banks free in ~0.7us.
                for h in range(2):
                    nc.scalar.copy(oh_sb[h][:, qsl], out_ps[h][:])
                    nc.vector.tensor_copy(den_sb[h][0:1, qsl],
                                          out_ps[h][D:D + 1, :])
                prev_qsl = qsl
            emit_proj(prev_qsl)

            for h in range(2):
                nc.sync.dma_start(den_d[h:h + 1, :], den_sb[h][0:1, :])

    nc.compile()
    return nc


def kernel(x, w_qkv, w_out, b_out):
    from concourse.bass_utils import run_bass_kernel_spmd
    global LAST_RESULTS

    if "nc" not in _CACHE:
        _CACHE["nc"] = _build()
    nc = _CACHE["nc"]

    x = np.ascontiguousarray(np.asarray(x, dtype=np.float32))
    w_qkv = np.asarray(w_qkv, dtype=np.float32)
    w_out = np.asarray(w_out, dtype=np.float32)
    b_out = np.asarray(b_out, dtype=np.float32)

    xf = x.reshape(B, C, HW)
    in_maps = []
    for core in range(N_CORES):
        bi, hp = divmod(core, 2)
        # rows of w_qkv for this core's two heads: q block then k block
        q_rows = w_qkv[0 * C + hp * 128: 0 * C + hp * 128 + 128]
        k_rows = w_qkv[1 * C + hp * 128: 1 * C + hp * 128 + 128]
        v_rows = w_qkv[2 * C + hp * 128: 2 * C + hp * 128 + 128]
        wqkT = np.concatenate([q_rows, k_rows], axis=0).T  # (256, 256)
        wvT = v_rows.T                                     # (256, 128)
        # woT: (64, 512): rows = head dim, cols = [h0 out-chans | h1]
        woT = np.concatenate(
            [w_out[:, hp * 128 + h * D: hp * 128 + (h + 1) * D].T
             for h in range(2)], axis=1)
        in_maps.append({
            "x": np.ascontiguousarray(xf[bi]).astype(_BF16),
            "wqkT": np.ascontiguousarray(wqkT).astype(_BF16),
            "wvT": np.ascontiguousarray(wvT).astype(_BF16),
            "woT": np.ascontiguousarray(woT).astype(_BF16),
        })

    trace = bool(int(os.environ.get("KERNEL_TRACE", "0")))
    print("kernel: program built, launching spmd run", flush=True)
    LAST_RESULTS = run_bass_kernel_spmd(
        nc, in_maps, core_ids=list(range(N_CORES)), trace=trace)

    out = np.empty((B, C, HW), dtype=np.float32)
    for bi in range(B):
        acc = xf[bi] + b_out[:, None]
        for hp in range(2):
            r = LAST_RESULTS.results[2 * bi + hp]
            den = r["den"]
            acc = acc + r["out0"] / den[0][None, :] + r["out1"] / den[1][None, :]
        out[bi] = acc
    return out.reshape(B, C, 64, 64)


# revision 16
# speedup vs baseline: 1.3835x; 1.0053x over previous
"""AttentionBlock kernel for 8 TRN2 NeuronCores.

Problem (hardcoded shapes): x (4, 256, 64, 64) f32, w_qkv (768, 256),
w_out (256, 256), b_out (256,). heads=4, d=64, seq=hw=4096.

Sharding: 16 independent (batch, head) attention units -> 8 cores,
core i handles batch i//2, head-pair i%2 (2 heads). Each core computes
its batch's qkv rows for its heads, flash-style attention (scores kept
transposed: j on partitions, q on free dim; softmax denominator via a
ones-column appended to V), and per-head partial output projections of
the UNNORMALIZED attention output. The softmax denominator rows ship to
the host, which applies the per-position division (it commutes with the
channel-mixing projection), sums partial projections, and adds
x + b_out.

All matmuls run in bf16 (f32 PSUM accumulate); softmax exp runs on
ScalarE in f32 from PSUM, grouped over 3 PSUM banks per instruction to
amortize ACT overhead. The two heads interleave per q-block so adjacent
score matmuls land on disjoint PE row groups and run concurrently.
Weights are pre-transposed/sliced on host so the device does no layout
fixups.
"""

import os
import sys
import types

import numpy as np
import ml_dtypes

# The agent image's antenv package lacks axon_hooks; the axon boot code
# degrades silently and run_bass_kernel_spmd(trace=True) then crashes on
# import. Pre-register the module so the boot can install the NTFF hook.
# Harmless when tracing is off.
if "antenv.axon_hooks" not in sys.modules:
    _m = types.ModuleType("antenv.axon_hooks")
    _m._hook = None

    def _set(h, _m=_m):
        _m._hook = h

    def _get(_m=_m):
        return _m._hook

    _m.set_axon_ntff_profile_hook = _set
    _m.get_axon_ntff_profile_hook = _get
    sys.modules["antenv.axon_hooks"] = _m
    # The axon boot (sitecustomize) runs before this module exists and
    # skips hook registration; re-derive the ctypes hook it would have
    # installed so trace=True can capture NTFF profiles.
    try:
        from trn_agent_boot.trn_boot import _ntff_profile_via_ctypes
        _m._hook = _ntff_profile_via_ctypes("/opt/axon/libaxon_pjrt.so")
    except Exception:
        pass

B = 4
C = 256
HW = 4096
HEADS = 4
D = 64
SCALE = D ** -0.5
N_CORES = 8
QB = 512          # q positions per block
NQB = HW // QB    # 8
JC = 128          # j positions per chunk (scores-matmul output partitions)
NJC = HW // JC    # 32
VROW = 2 * (D + 1)  # per-j-chunk v layout: [v_h0(64) | 1 | v_h1(64) | 1]

_BF16 = ml_dtypes.bfloat16

_CACHE = {}
LAST_RESULTS = None


def _build():
    import concourse.bass as bass
    import concourse.tile as tile
    from concourse import bacc, mybir

    f32 = mybir.dt.float32
    bf16 = mybir.dt.bfloat16
    Exp = mybir.ActivationFunctionType.Exp

    nc = bacc.Bacc("TRN2", target_bir_lowering=False, debug=False,
                   enable_asserts=False)

    x_d = nc.dram_tensor("x", [C, HW], bf16, kind="ExternalInput").ap()
    wqkT_d = nc.dram_tensor("wqkT", [C, 2 * 128], bf16, kind="ExternalInput").ap()
    wvT_d = nc.dram_tensor("wvT", [C, 128], bf16, kind="ExternalInput").ap()
    # woT rows: head dim d (64); cols: [h0 out-chans (256) | h1 out-chans]
    woT_d = nc.dram_tensor("woT", [D, 2 * C], bf16, kind="ExternalInput").ap()
    out0_d = nc.dram_tensor("out0", [C, HW], f32, kind="ExternalOutput").ap()
    out1_d = nc.dram_tensor("out1", [C, HW], f32, kind="ExternalOutput").ap()
    den_d = nc.dram_tensor("den", [2, HW], f32, kind="ExternalOutput").ap()

    with tile.TileContext(nc) as tc:
        with (
            tc.tile_pool(name="big", bufs=1) as big,
            tc.tile_pool(name="attn", bufs=3) as attnp,
            tc.tile_pool(name="small", bufs=2) as small,
            tc.tile_pool(name="psc", bufs=2, space="PSUM") as psc,
            tc.tile_pool(name="pout", bufs=1, space="PSUM") as pout,
        ):
            # ---- load inputs ----
            xb = []
            for kc in range(2):
                t = big.tile([128, HW], bf16, name=f"xb{kc}", tag=f"xb{kc}")
                nc.sync.dma_start(t[:], x_d[kc * 128:(kc + 1) * 128, :])
                xb.append(t)
            wqkT = []
            for kc in range(2):
                t = big.tile([128, 256], bf16, name=f"wqkT{kc}", tag=f"wqkT{kc}")
                nc.sync.dma_start(t[:], wqkT_d[kc * 128:(kc + 1) * 128, :])
                wqkT.append(t)
            wvT = []
            for kc in range(2):
                t = big.tile([128, 128], bf16, name=f"wvT{kc}", tag=f"wvT{kc}")
                nc.sync.dma_start(t[:], wvT_d[kc * 128:(kc + 1) * 128, :])
                wvT.append(t)
            woT = big.tile([D, 2 * C], bf16, name="woT", tag="woT")
            nc.sync.dma_start(woT[:], woT_d[:, :])

            q_sb = big.tile([128, HW], bf16, name="q_sb", tag="q_sb")
            k_sb = big.tile([128, HW], bf16, name="k_sb", tag="k_sb")
            v_sb = big.tile([128, NJC * VROW], bf16, name="v_sb", tag="v_sb")
            # unnormalized per-head attention output (d on partitions),
            # row 64 carries the softmax denominator (unused by proj)
            oh_sb = [big.tile([D + 1, HW], bf16, name=f"oh{h}", tag=f"oh{h}")
                     for h in range(2)]
            den_sb = [big.tile([1, HW], f32, name=f"den_sb{h}",
                               tag=f"den_sb{h}") for h in range(2)]

            # ---- qkv projections ----
            # q_sb/k_sb: (2 heads * 64 chan, pos);  m=0 -> q rows, m=1 -> k
            for m in range(2):
                dest = q_sb if m == 0 else k_sb
                for nb in range(NQB):
                    ps = psc.tile([128, QB], f32, name="ps_qk", tag="psc")
                    for kc in range(2):
                        nc.tensor.matmul(
                            ps[:],
                            lhsT=wqkT[kc][:, m * 128:(m + 1) * 128],
                            rhs=xb[kc][:, nb * QB:(nb + 1) * QB],
                            start=(kc == 0), stop=(kc == 1),
                        )
                    nc.vector.tensor_copy(dest[:, nb * QB:(nb + 1) * QB], ps[:])

            # v transposed: per j-chunk (128 pos, [v_h0|1|v_h1|1])
            nc.vector.memset(v_sb[:], 1.0)
            for pc in range(NJC):
                ps = psc.tile([128, 128], f32, name="ps_v", tag="psc")
                for kc in range(2):
                    nc.tensor.matmul(
                        ps[:],
                        lhsT=xb[kc][:, pc * 128:(pc + 1) * 128],
                        rhs=wvT[kc][:],
                        start=(kc == 0), stop=(kc == 1),
                    )
                base = pc * VROW
                nc.vector.tensor_copy(v_sb[:, base:base + D], ps[:, 0:D])
                nc.vector.tensor_copy(
                    v_sb[:, base + D + 1:base + 2 * D + 1], ps[:, D:2 * D])

            # ---- attention ----
            # Heads interleaved per q-block: adjacent score matmuls use
            # disjoint PE row groups (h0 rows 0-63, h1 rows 64-127) and run
            # concurrently. Stream index s -> (j, h) = (s // 2, s % 2).
            NS = 2 * NJC

            def emit_proj(qsl):
                # partial projection of a finished q-block; psum from the
                # pout slots (free between q-blocks)
                for h in range(2):
                    od = out0_d if h == 0 else out1_d
                    for m in range(2):
                        ps = psc.tile([128, QB], f32, name=f"ps_pr{h}",
                                      tag="psc")
                        nc.tensor.matmul(
                            ps[:],
                            lhsT=woT[:, h * C + m * 128:h * C + (m + 1) * 128],
                            rhs=oh_sb[h][0:D, qsl],
                            start=True, stop=True,
                        )
                        st = small.tile([128, QB], f32, name="st", tag="st")
                        nc.vector.tensor_copy(st[:], ps[:])
                        nc.sync.dma_start(od[m * 128:(m + 1) * 128, qsl], st[:])

            prev_qsl = None
            for qb in range(NQB):
                qsl = slice(qb * QB, (qb + 1) * QB)
                out_ps = [
                    pout.tile([D + 1, QB], f32, name=f"out_ps{h}",
                              tag=f"pout{h}")
                    for h in range(2)
                ]
                s = 0
                while s < NS:
                    gsz = min(3, NS - s)
                    s_ps = psc.tile([128, 3 * QB], f32, name="s_ps", tag="psc")
                    for t in range(gsz):
                        j, h = divmod(s + t, 2)
                        hp = h * D
                        nc.tensor.matmul(
                            s_ps[:, t * QB:(t + 1) * QB],
                            lhsT=k_sb[hp:hp + D, j * JC:(j + 1) * JC],
                            rhs=q_sb[hp:hp + D, qsl],
                            start=True, stop=True,
                        )
                    a_sb = attnp.tile([128, 3 * QB], bf16, name="a_sb",
                                      tag="attn")
                    nc.scalar.activation(
                        a_sb[:, 0:gsz * QB], s_ps[:, 0:gsz * QB],
                        Exp, scale=SCALE)
                    for t in range(gsz):
                        j, h = divmod(s + t, 2)
                        vo = h * (D + 1)
                        nc.tensor.matmul(
                            out_ps[h][:],
                            lhsT=v_sb[:, j * VROW + vo:j * VROW + vo + D + 1],
                            rhs=a_sb[:, t * QB:(t + 1) * QB],
                            start=(j == 0), stop=(j == NJC - 1),
                        )
                    s += gsz
                # ship unnormalized output + denominator. The 65-row copy
                # runs on ScalarE (idle at block boundaries, fast PSUM port)
                # and the f32 denominator copy on VectorE in parallel, so
                # the accumulator banks free in ~0.7us.
                for h in range(2):
                    nc.scalar.copy(oh_sb[h][:, qsl], out_ps[h][:])
                    nc.vector.tensor_copy(den_sb[h][0:1, qsl],
                                          out_ps[h][D:D + 1, :])
                prev_qsl = qsl
            # all projections in one epilogue: boundary stalls in the main
            # loop cost more (ACT starvation + PE re-throttle) than a short
            # serial tail here
            for qb in range(NQB):
                emit_proj(slice(qb * QB, (qb + 1) * QB))

            for h in range(2):
                nc.sync.dma_start(den_d[h:h + 1, :], den_sb[h][0:1, :])

    nc.compile()
    return nc


def kernel(x, w_qkv, w_out, b_out):
    from concourse.bass_utils import run_bass_kernel_spmd
    global LAST_RESULTS

    if "nc" not in _CACHE:
        _CACHE["nc"] = _build()
    nc = _CACHE["nc"]

    x = np.ascontiguousarray(np.asarray(x, dtype=np.float32))
    w_qkv = np.asarray(w_qkv, dtype=np.float32)
    w_out = np.asarray(w_out, dtype=np.float32)
    b_out = np.asarray(b_out, dtype=np.float32)

    xf = x.reshape(B, C, HW)
    in_maps = []
    for core in range(N_CORES):
        bi, hp = divmod(core, 2)
        # rows of w_qkv for this core's two heads: q block then k block
        q_rows = w_qkv[0 * C + hp * 128: 0 * C + hp * 128 + 128]
        k_rows = w_qkv[1 * C + hp * 128: 1 * C + hp * 128 + 128]
        v_rows = w_qkv[2 * C + hp * 128: 2 * C + hp * 128 + 128]
        wqkT = np.concatenate([q_rows, k_rows], axis=0).T  # (256, 256)
        wvT = v_rows.T                                     # (256, 128)
        # woT: (64, 512): rows = head dim, cols = [h0 out-chans | h1]
        woT = np.concatenate(
            [w_out[:, hp * 128 + h * D: hp * 128 + (h + 1) * D].T
             for h in range(2)], axis=1)
        in_maps.append({
            "x": np.ascontiguousarray(xf[bi]).astype(_BF16),
            "wqkT": np.ascontiguousarray(wqkT).astype(_BF16),
            "wvT": np.ascontiguousarray(wvT).astype(_BF16),
            "woT": np.ascontiguousarray(woT).astype(_BF16),
        })

    trace = bool(int(os.environ.get("KERNEL_TRACE", "0")))
    print("kernel: program built, launching spmd run", flush=True)
    LAST_RESULTS = run_bass_kernel_spmd(
        nc, in_maps, core_ids=list(range(N_CORES)), trace=trace)

    out = np.empty((B, C, HW), dtype=np.float32)
    for bi in range(B):
        acc = xf[bi] + b_out[:, None]
        for hp in range(2):
            r = LAST_RESULTS.results[2 * bi + hp]
            den = r["den"]
            acc = acc + r["out0"] / den[0][None, :] + r["out1"] / den[1][None, :]
        out[bi] = acc
    return out.reshape(B, C, 64, 64)


# revision 17
# speedup vs baseline: 1.4588x; 1.0544x over previous
"""AttentionBlock kernel for 8 TRN2 NeuronCores.

Problem (hardcoded shapes): x (4, 256, 64, 64) f32, w_qkv (768, 256),
w_out (256, 256), b_out (256,). heads=4, d=64, seq=hw=4096.

Sharding: 16 independent (batch, head) attention units -> 8 cores,
core i handles batch i//2, head-pair i%2 (2 heads). Each core computes
its batch's qkv rows for its heads, flash-style attention (scores kept
transposed: j on partitions, q on free dim; softmax denominator via a
ones-column appended to V), and per-head partial output projections of
the UNNORMALIZED attention output. The softmax denominator rows ship to
the host, which applies the per-position division (it commutes with the
channel-mixing projection), sums partial projections, and adds
x + b_out.

All matmuls run in bf16 (f32 PSUM accumulate); softmax exp runs on
ScalarE in f32 from PSUM, grouped over 3 PSUM banks per instruction to
amortize ACT overhead. The two heads interleave per q-block so adjacent
score matmuls land on disjoint PE row groups and run concurrently.
Weights are pre-transposed/sliced on host so the device does no layout
fixups.
"""

import os
import sys
import types

import numpy as np
import ml_dtypes

# The agent image's antenv package lacks axon_hooks; the axon boot code
# degrades silently and run_bass_kernel_spmd(trace=True) then crashes on
# import. Pre-register the module so the boot can install the NTFF hook.
# Harmless when tracing is off.
if "antenv.axon_hooks" not in sys.modules:
    _m = types.ModuleType("antenv.axon_hooks")
    _m._hook = None

    def _set(h, _m=_m):
        _m._hook = h

    def _get(_m=_m):
        return _m._hook

    _m.set_axon_ntff_profile_hook = _set
    _m.get_axon_ntff_profile_hook = _get
    sys.modules["antenv.axon_hooks"] = _m
    # The axon boot (sitecustomize) runs before this module exists and
    # skips hook registration; re-derive the ctypes hook it would have
    # installed so trace=True can capture NTFF profiles.
    try:
        from trn_agent_boot.trn_boot import _ntff_profile_via_ctypes
        _m._hook = _ntff_profile_via_ctypes("/opt/axon/libaxon_pjrt.so")
    except Exception:
        pass

B = 4
C = 256
HW = 4096
HEADS = 4
D = 64
SCALE = D ** -0.5
N_CORES = 8
QB = 512          # q positions per block
NQB = HW // QB    # 8
JC = 128          # j positions per chunk (scores-matmul output partitions)
NJC = HW // JC    # 32
VROW = 2 * (D + 1)  # per-j-chunk v layout: [v_h0(64) | 1 | v_h1(64) | 1]

_BF16 = ml_dtypes.bfloat16

_CACHE = {}
LAST_RESULTS = None


def _build():
    import concourse.bass as bass
    import concourse.tile as tile
    from concourse import bacc, mybir

    f32 = mybir.dt.float32
    bf16 = mybir.dt.bfloat16
    Exp = mybir.ActivationFunctionType.Exp

    nc = bacc.Bacc("TRN2", target_bir_lowering=False, debug=False,
                   enable_asserts=False)

    x_d = nc.dram_tensor("x", [C, HW], bf16, kind="ExternalInput").ap()
    wqkT_d = nc.dram_tensor("wqkT", [C, 2 * 128], bf16, kind="ExternalInput").ap()
    wvT_d = nc.dram_tensor("wvT", [C, 128], bf16, kind="ExternalInput").ap()
    # woT rows: head dim d (64); cols: [h0 out-chans (256) | h1 out-chans]
    woT_d = nc.dram_tensor("woT", [D, 2 * C], bf16, kind="ExternalInput").ap()
    out0_d = nc.dram_tensor("out0", [C, HW], f32, kind="ExternalOutput").ap()
    out1_d = nc.dram_tensor("out1", [C, HW], f32, kind="ExternalOutput").ap()
    den_d = nc.dram_tensor("den", [2, HW], f32, kind="ExternalOutput").ap()

    with tile.TileContext(nc) as tc:
        with (
            tc.tile_pool(name="big", bufs=1) as big,
            tc.tile_pool(name="attn", bufs=3) as attnp,
            tc.tile_pool(name="small", bufs=2) as small,
            tc.tile_pool(name="psc", bufs=2, space="PSUM") as psc,
            tc.tile_pool(name="pout", bufs=1, space="PSUM") as pout,
        ):
            # ---- load inputs ----
            xb = []
            for kc in range(2):
                t = big.tile([128, HW], bf16, name=f"xb{kc}", tag=f"xb{kc}")
                nc.sync.dma_start(t[:], x_d[kc * 128:(kc + 1) * 128, :])
                xb.append(t)
            wqkT = []
            for kc in range(2):
                t = big.tile([128, 256], bf16, name=f"wqkT{kc}", tag=f"wqkT{kc}")
                nc.sync.dma_start(t[:], wqkT_d[kc * 128:(kc + 1) * 128, :])
                wqkT.append(t)
            wvT = []
            for kc in range(2):
                t = big.tile([128, 128], bf16, name=f"wvT{kc}", tag=f"wvT{kc}")
                nc.sync.dma_start(t[:], wvT_d[kc * 128:(kc + 1) * 128, :])
                wvT.append(t)
            woT = big.tile([D, 2 * C], bf16, name="woT", tag="woT")
            nc.sync.dma_start(woT[:], woT_d[:, :])

            q_sb = big.tile([128, HW], bf16, name="q_sb", tag="q_sb")
            k_sb = big.tile([128, HW], bf16, name="k_sb", tag="k_sb")
            v_sb = big.tile([128, NJC * VROW], bf16, name="v_sb", tag="v_sb")
            # unnormalized per-head attention output (d on partitions),
            # row 64 carries the softmax denominator (unused by proj)
            oh_sb = [big.tile([D + 1, HW], bf16, name=f"oh{h}", tag=f"oh{h}")
                     for h in range(2)]
            den_sb = [big.tile([1, HW], f32, name=f"den_sb{h}",
                               tag=f"den_sb{h}") for h in range(2)]

            # ---- qkv projections ----
            # q_sb/k_sb: (2 heads * 64 chan, pos);  m=0 -> q rows, m=1 -> k
            for m in range(2):
                dest = q_sb if m == 0 else k_sb
                for nb in range(NQB):
                    ps = psc.tile([128, QB], f32, name="ps_qk", tag="psc")
                    for kc in range(2):
                        nc.tensor.matmul(
                            ps[:],
                            lhsT=wqkT[kc][:, m * 128:(m + 1) * 128],
                            rhs=xb[kc][:, nb * QB:(nb + 1) * QB],
                            start=(kc == 0), stop=(kc == 1),
                        )
                    nc.vector.tensor_copy(dest[:, nb * QB:(nb + 1) * QB], ps[:])

            # v transposed: per j-chunk (128 pos, [v_h0|1|v_h1|1])
            nc.vector.memset(v_sb[:], 1.0)
            for pc in range(NJC):
                ps = psc.tile([128, 128], f32, name="ps_v", tag="psc")
                for kc in range(2):
                    nc.tensor.matmul(
                        ps[:],
                        lhsT=xb[kc][:, pc * 128:(pc + 1) * 128],
                        rhs=wvT[kc][:],
                        start=(kc == 0), stop=(kc == 1),
                    )
                base = pc * VROW
                nc.vector.tensor_copy(v_sb[:, base:base + D], ps[:, 0:D])
                nc.vector.tensor_copy(
                    v_sb[:, base + D + 1:base + 2 * D + 1], ps[:, D:2 * D])

            # ---- attention ----
            # Heads interleaved per q-block: adjacent score matmuls use
            # disjoint PE row groups (h0 rows 0-63, h1 rows 64-127) and run
            # concurrently. Stream index s -> (j, h) = (s // 2, s % 2).
            NS = 2 * NJC

            def alloc_proj_tiles():
                return [pout.tile([128, QB], f32, name=f"ps_pr{h}",
                                  tag=f"pout{h}")
                        for h in range(2) for _ in range(2)]

            def emit_proj(qsl, tiles):
                # partial projection of a finished q-block, into pre-reserved
                # pout slots so the score-stream PSUM banks are untouched
                for h in range(2):
                    od = out0_d if h == 0 else out1_d
                    for m in range(2):
                        ps = tiles[2 * h + m]
                        nc.tensor.matmul(
                            ps[:],
                            lhsT=woT[:, h * C + m * 128:h * C + (m + 1) * 128],
                            rhs=oh_sb[h][0:D, qsl],
                            start=True, stop=True,
                        )
                        st = small.tile([128, QB], f32, name="st", tag="st")
                        nc.vector.tensor_copy(st[:], ps[:])
                        nc.sync.dma_start(od[m * 128:(m + 1) * 128, qsl], st[:])

            pending = None
            for qb in range(NQB):
                qsl = slice(qb * QB, (qb + 1) * QB)
                # reserve proj psum slots for the 2-blocks-ago projection
                # BEFORE this block's accumulators so slot order is correct
                if qb >= 2:
                    pending = (slice((qb - 2) * QB, (qb - 1) * QB),
                               alloc_proj_tiles())
                out_ps = [
                    pout.tile([D + 1, QB], f32, name=f"out_ps{h}",
                              tag=f"pout{h}")
                    for h in range(2)
                ]
                s = 0
                while s < NS:
                    gsz = min(3, NS - s)
                    s_ps = psc.tile([128, 3 * QB], f32, name="s_ps", tag="psc")
                    for t in range(gsz):
                        j, h = divmod(s + t, 2)
                        hp = h * D
                        nc.tensor.matmul(
                            s_ps[:, t * QB:(t + 1) * QB],
                            lhsT=k_sb[hp:hp + D, j * JC:(j + 1) * JC],
                            rhs=q_sb[hp:hp + D, qsl],
                            start=True, stop=True,
                        )
                    a_sb = attnp.tile([128, 3 * QB], bf16, name="a_sb",
                                      tag="attn")
                    nc.scalar.activation(
                        a_sb[:, 0:gsz * QB], s_ps[:, 0:gsz * QB],
                        Exp, scale=SCALE)
                    for t in range(gsz):
                        j, h = divmod(s + t, 2)
                        vo = h * (D + 1)
                        nc.tensor.matmul(
                            out_ps[h][:],
                            lhsT=v_sb[:, j * VROW + vo:j * VROW + vo + D + 1],
                            rhs=a_sb[:, t * QB:(t + 1) * QB],
                            start=(j == 0), stop=(j == NJC - 1),
                        )
                    s += gsz
                    if s == 3 and pending is not None:
                        emit_proj(*pending)
                        pending = None
                # ship unnormalized output + denominator. The 65-row copy
                # runs on ScalarE (idle at block boundaries, fast PSUM port)
                # and the f32 denominator copy on VectorE in parallel, so
                # the accumulator banks free in ~0.7us.
                for h in range(2):
                    nc.scalar.copy(oh_sb[h][:, qsl], out_ps[h][:])
                    nc.vector.tensor_copy(den_sb[h][0:1, qsl],
                                          out_ps[h][D:D + 1, :])
            # last two q-blocks' projections in the epilogue
            for qb in range(NQB - 2, NQB):
                emit_proj(slice(qb * QB, (qb + 1) * QB), alloc_proj_tiles())

            for h in range(2):
                nc.sync.dma_start(den_d[h:h + 1, :], den_sb[h][0:1, :])

    nc.compile()
    return nc


def kernel(x, w_qkv, w_out, b_out):
    from concourse.bass_utils import run_bass_kernel_spmd
    global LAST_RESULTS

    if "nc" not in _CACHE:
        _CACHE["nc"] = _build()
    nc = _CACHE["nc"]

    x = np.ascontiguousarray(np.asarray(x, dtype=np.float32))
    w_qkv = np.asarray(w_qkv, dtype=np.float32)
    w_out = np.asarray(w_out, dtype=np.float32)
    b_out = np.asarray(b_out, dtype=np.float32)

    xf = x.reshape(B, C, HW)
    in_maps = []
    for core in range(N_CORES):
        bi, hp = divmod(core, 2)
        # rows of w_qkv for this core's two heads: q block then k block
        q_rows = w_qkv[0 * C + hp * 128: 0 * C + hp * 128 + 128]
        k_rows = w_qkv[1 * C + hp * 128: 1 * C + hp * 128 + 128]
        v_rows = w_qkv[2 * C + hp * 128: 2 * C + hp * 128 + 128]
        wqkT = np.concatenate([q_rows, k_rows], axis=0).T  # (256, 256)
        wvT = v_rows.T                                     # (256, 128)
        # woT: (64, 512): rows = head dim, cols = [h0 out-chans | h1]
        woT = np.concatenate(
            [w_out[:, hp * 128 + h * D: hp * 128 + (h + 1) * D].T
             for h in range(2)], axis=1)
        in_maps.append({
            "x": np.ascontiguousarray(xf[bi]).astype(_BF16),
            "wqkT": np.ascontiguousarray(wqkT).astype(_BF16),
            "wvT": np.ascontiguousarray(wvT).astype(_BF16),
            "woT": np.ascontiguousarray(woT).astype(_BF16),
        })

    trace = bool(int(os.environ.get("KERNEL_TRACE", "0")))
    print("kernel: program built, launching spmd run", flush=True)
    LAST_RESULTS = run_bass_kernel_spmd(
        nc, in_maps, core_ids=list(range(N_CORES)), trace=trace)

    out = np.empty((B, C, HW), dtype=np.float32)
    for bi in range(B):
        acc = xf[bi] + b_out[:, None]
        for hp in range(2):
            r = LAST_RESULTS.results[2 * bi + hp]
            den = r["den"]
            acc = acc + r["out0"] / den[0][None, :] + r["out1"] / den[1][None, :]
        out[bi] = acc
    return out.reshape(B, C, 64, 64)
